# revision 1
# baseline (speedup 1.0000x reference)
"""GATv2 (nn_GATv2_59184649339075) Bass kernel for TRN2, 8-core SPMD.

Self-contained: kernel(**inputs) takes the full unsharded inputs
(x[50000,64], W[64,64], b[64], a[64], edge_index[2,800000] int32) and
returns the full [50000,64] float32 output.

Strategy (edge-parallel with dst-tile ownership, no collectives):
  - Host: pad nodes to 51200 (400 tiles of 128). Sort edges by dst tile;
    each core owns 50 consecutive dst tiles (node-range sharded output).
    Within a tile, split edges into A (src < 25600) / B (src >= 25600) so
    dma_gather int16 indices cover the Wh table; sort each group by src
    for HBM locality; pad each group to fixed per-tile slot counts
    (compile-time max over all cores/tiles) so the SPMD program is static.
  - Device per core: Wh = x@W.T + b on-chip (bias folded in as an
    augmented contraction row), written to a DRAM table (256B rows) plus
    a per-core slice table for dst-side gathers.
    dma_gather (SWDGE, 4 queues round-robin, multi-packet) fetches Wh rows
    per edge by src (A/B calls) and by core-local dst.
    Edge-major score pipeline: s = Wh_dst+Wh_src (DVE), LeakyReLU (ACT
    Prelu alpha=0.2), *a + reduce (DVE), exp (ACT).
    One-hot [128e x 128n] f16 built via is_equal against an iota row;
    PE matmul onehot.T @ [exp*Wh_src | exp] (f16) accumulates numerator
    and denominator [128n, 65] in PSUM per dst tile.
    Output: sigmoid(numer * 1/denom) via ACT with per-partition scale.
"""
import sys

sys.path.insert(0, "/opt/trn_rl_repo")
from contextlib import ExitStack
from dataclasses import dataclass

import numpy as np

import concourse.bass as bass
import concourse.tile as tile
from concourse import bacc, mybir

F32 = mybir.dt.float32
F16 = mybir.dt.float16
I16 = mybir.dt.int16
I32 = mybir.dt.int32
AF = mybir.ActivationFunctionType

N_CORES = 8
P = 128
DIN = 64
DOUT = 64
NSLOPE = 0.2


@dataclass(frozen=True)
class GatCfg:
    n_pad: int          # padded node count, multiple of 128*8*gb_tiles
    sa: int             # per-tile A slots (multiple of 128)
    sb: int             # per-tile B slots
    gb_tiles: int       # tiles per gather super-batch

    @property
    def n_loc(self):
        return self.n_pad // N_CORES

    @property
    def tiles_core(self):
        return self.n_loc // P

    @property
    def half(self):
        return self.n_pad // 2

    @property
    def ca(self):
        return self.sa // P

    @property
    def cb(self):
        return self.sb // P

    @property
    def ct(self):
        return self.ca + self.cb

    @property
    def c_tot(self):
        return self.tiles_core * self.ct


def wrap16(idx):
    """Slot i of a gather call -> idx array position [i%16, i//16],
    replicated to the 128 partitions."""
    n = len(idx)
    assert n % 16 == 0
    a = idx.reshape(n // 16, 16).T.astype(np.int16)
    return np.tile(a, (8, 1))


def prepare(x, W, b, a, edge_index, gb_tiles=2):
    N = x.shape[0]
    E = edge_index.shape[1]
    blk = P * N_CORES * gb_tiles
    n_pad = ((N + blk - 1) // blk) * blk
    src = edge_index[0].astype(np.int64)
    dst = edge_index[1].astype(np.int64)
    half = n_pad // 2
    assert half <= 32768, "int16 gather indices require n_pad <= 65536"

    tile_id = dst >> 7
    grp = (src >= half).astype(np.int64)
    order = np.lexsort((src, grp, tile_id))
    src_s, dst_s, tile_s, grp_s = src[order], dst[order], tile_id[order], grp[order]

    n_tiles = n_pad // P
    tiles_core = n_tiles // N_CORES
    key = tile_s * 2 + grp_s
    counts = np.bincount(key, minlength=n_tiles * 2).reshape(n_tiles, 2)
    sa = max(int(np.ceil(counts[:, 0].max() / P) * P), P)
    sb = max(int(np.ceil(counts[:, 1].max() / P) * P), P)
    cfg = GatCfg(n_pad=n_pad, sa=sa, sb=sb, gb_tiles=gb_tiles)
    assert cfg.tiles_core % cfg.gb_tiles == 0

    rank_in_grp = np.arange(E) - np.repeat(
        np.concatenate([[0], np.cumsum(counts.reshape(-1))[:-1]]), counts.reshape(-1))
    core_of = tile_s // tiles_core
    t_in_core = tile_s % tiles_core
    batch = t_in_core // cfg.gb_tiles
    t_in_b = t_in_core % cfg.gb_tiles
    gb = cfg.gb_tiles
    batch_slots = gb * (sa + sb)
    slot = (batch * batch_slots
            + np.where(grp_s == 0,
                       t_in_b * sa + rank_in_grp,
                       gb * sa + t_in_b * sb + rank_in_grp))

    slots_core = tiles_core * (sa + sb)
    n_loc = cfg.n_loc
    n_loc_w = ((tiles_core + 7) // 8) * 8 * P

    xT = np.zeros((DIN + 1, n_pad), np.float32)
    xT[:DIN, :N] = x.T
    xT[DIN, :] = 1.0
    WT = np.concatenate([W.T.astype(np.float32),
                         b.reshape(1, DOUT).astype(np.float32)])
    a_row = a.reshape(1, DOUT).astype(np.float32)

    n_batches = tiles_core // gb
    fa_b = gb * sa // 16
    fb_b = gb * sb // 16
    fd_b = gb * (sa + sb) // 16

    in_maps = []
    for c in range(N_CORES):
        m = core_of == c
        s_src, s_dst, s_slot, s_grp = src_s[m], dst_s[m], slot[m], grp_s[m]
        srcA = np.zeros(slots_core, np.int16)
        srcB = np.zeros(slots_core, np.int16)
        dstL = np.zeros(slots_core, np.int16)
        dtl = np.full(slots_core, -1.0, np.float32)
        srcA[s_slot[s_grp == 0]] = s_src[s_grp == 0].astype(np.int16)
        srcB[s_slot[s_grp == 1]] = (s_src[s_grp == 1] - half).astype(np.int16)
        dstL[s_slot] = (s_dst - c * n_loc).astype(np.int16)
        dtl[s_slot] = (s_dst & (P - 1)).astype(np.float32)

        srcA_w = np.zeros((P, n_batches * fa_b), np.int16)
        srcB_w = np.zeros((P, n_batches * fb_b), np.int16)
        dstL_w = np.zeros((P, n_batches * fd_b), np.int16)
        for i in range(n_batches):
            lo = i * batch_slots
            srcA_w[:, i * fa_b:(i + 1) * fa_b] = wrap16(srcA[lo:lo + gb * sa])
            srcB_w[:, i * fb_b:(i + 1) * fb_b] = wrap16(
                srcB[lo + gb * sa:lo + batch_slots])
            dstL_w[:, i * fd_b:(i + 1) * fd_b] = wrap16(dstL[lo:lo + batch_slots])
        dtl_w = np.ascontiguousarray(dtl.reshape(cfg.c_tot, P).T)

        in_maps.append({
            "xT": xT,
            "xTs": np.ascontiguousarray(
                np.pad(xT[:, c * n_loc:(c + 1) * n_loc],
                       ((0, 0), (0, n_loc_w - n_loc)))),
            "WT": WT, "a": a_row,
            "srcA": srcA_w, "srcB": srcB_w, "dstL": dstL_w, "dtl": dtl_w,
        })
    return cfg, in_maps, {"N": N, "cfg": cfg}


def build(cfg: GatCfg, reps=1):
    nc = bacc.Bacc("TRN2", target_bir_lowering=False, debug=False,
                   num_devices=N_CORES, num_swdge_queues=4)
    n_pad, n_loc = cfg.n_pad, cfg.n_loc
    gb, sa, sb = cfg.gb_tiles, cfg.sa, cfg.sb
    ca, cb = cfg.ca, cfg.cb
    tiles_core = cfg.tiles_core
    n_batches = tiles_core // gb
    bc = gb * cfg.ct
    fa_b = gb * sa // 16
    fb_b = gb * sb // 16
    fd_b = gb * (sa + sb) // 16
    n_loc_w = ((tiles_core + 7) // 8) * 8 * P

    xT_d = nc.dram_tensor("xT", [DIN + 1, n_pad], F32, kind="ExternalInput").ap()
    xTs_d = nc.dram_tensor("xTs", [DIN + 1, n_loc_w], F32, kind="ExternalInput").ap()
    WT_d = nc.dram_tensor("WT", [DIN + 1, DOUT], F32, kind="ExternalInput").ap()
    a_d = nc.dram_tensor("a", [1, DOUT], F32, kind="ExternalInput").ap()
    srcA_d = nc.dram_tensor("srcA", [P, n_batches * fa_b], I16, kind="ExternalInput").ap()
    srcB_d = nc.dram_tensor("srcB", [P, n_batches * fb_b], I16, kind="ExternalInput").ap()
    dstL_d = nc.dram_tensor("dstL", [P, n_batches * fd_b], I16, kind="ExternalInput").ap()
    dtl_d = nc.dram_tensor("dtl", [P, cfg.c_tot], F32, kind="ExternalInput").ap()
    out_d = nc.dram_tensor("out", [n_loc, DOUT], F32, kind="ExternalOutput").ap()
    wh_d = nc.dram_tensor("wh", [n_pad, DOUT], F32).ap()
    whs_d = nc.dram_tensor("whs", [n_loc_w, DOUT], F32).ap()

    with tile.TileContext(nc) as tc:
        with ExitStack() as ctx:
            cpool = ctx.enter_context(tc.tile_pool(name="const", bufs=1))
            WT_sb = cpool.tile([DIN + 1, DOUT], F32)
            nc.sync.dma_start(WT_sb[:], WT_d[:, :])
            a_rep = cpool.tile([P, DOUT], F32)
            nc.sync.dma_start(a_rep[:], a_d.to_broadcast((P, DOUT)))
            iota_i = cpool.tile([P, P], I32)
            nc.gpsimd.iota(iota_i[:], pattern=[[1, P]], base=0, channel_multiplier=0)
            iota_f = cpool.tile([P, P], F32)
            nc.vector.tensor_copy(iota_f[:], iota_i[:])

            def wh_stage(ctx, src_ap, dst_ap, n_t):
                xp = ctx.enter_context(tc.tile_pool(name="xt", bufs=3))
                pp = ctx.enter_context(tc.tile_pool(name="whps", bufs=4, space="PSUM"))
                wp = ctx.enter_context(tc.tile_pool(name="whsb", bufs=3))
                GT = 8
                assert n_t % GT == 0
                for g in range(n_t // GT):
                    t0 = g * GT
                    xt = xp.tile([DIN + 1, GT * P], F32, tag="xt")
                    nc.sync.dma_start(xt[:], src_ap[:, t0 * P:(t0 + GT) * P])
                    ps = pp.tile([P, GT, DOUT], F32, tag="ps")
                    for j in range(GT):
                        nc.tensor.matmul(ps[:, j, :], lhsT=xt[:, j * P:(j + 1) * P],
                                         rhs=WT_sb[:], start=True, stop=True)
                    whb = wp.tile([P, GT, DOUT], F32, tag="whb")
                    nc.vector.tensor_copy(whb[:], ps[:])
                    nc.sync.dma_start(
                        dst_ap[t0 * P:(t0 + GT) * P, :]
                        .rearrange("(g p) f -> p g f", p=P), whb[:])

            with ExitStack() as c2:
                wh_stage(c2, xT_d, wh_d, n_pad // P)
                wh_stage(c2, xTs_d, whs_d, n_loc_w // P)

            gpool = ctx.enter_context(tc.tile_pool(name="gather", bufs=2))
            ipool = ctx.enter_context(tc.tile_pool(name="idx", bufs=2))
            spool = ctx.enter_context(tc.tile_pool(name="score", bufs=2))
            vpool = ctx.enter_context(tc.tile_pool(name="vals", bufs=2))
            opool = ctx.enter_context(tc.tile_pool(name="oh", bufs=2))
            apool = ctx.enter_context(tc.tile_pool(name="agg", bufs=4, space="PSUM"))
            npool = ctx.enter_context(tc.tile_pool(name="norm", bufs=4))

            for _ in range(reps):
                for bi in range(n_batches):
                    idxA = ipool.tile([P, fa_b], I16, tag="ia")
                    nc.sync.dma_start(idxA[:], srcA_d[:, bi * fa_b:(bi + 1) * fa_b])
                    idxB = ipool.tile([P, fb_b], I16, tag="ib")
                    nc.sync.dma_start(idxB[:], srcB_d[:, bi * fb_b:(bi + 1) * fb_b])
                    idxD = ipool.tile([P, fd_b], I16, tag="id")
                    nc.sync.dma_start(idxD[:], dstL_d[:, bi * fd_b:(bi + 1) * fd_b])
                    dtl = ipool.tile([P, bc], F32, tag="dtl")
                    nc.sync.dma_start(dtl[:], dtl_d[:, bi * bc:(bi + 1) * bc])

                    whsrc = gpool.tile([P, bc, DOUT], F32, tag="whsrc")
                    whdst = gpool.tile([P, bc, DOUT], F32, tag="whdst")
                    nA = gb * ca
                    q0 = (3 * bi) % 4
                    nc.gpsimd.dma_gather(
                        out_ap=whsrc[:, 0:nA, :], in_ap=wh_d[0:cfg.half, :],
                        idxs_ap=idxA[:], num_idxs=nA * P, num_idxs_reg=nA * P,
                        elem_size=DOUT, single_packet=False, queue_num=q0)
                    nc.gpsimd.dma_gather(
                        out_ap=whsrc[:, nA:bc, :], in_ap=wh_d[cfg.half:n_pad, :],
                        idxs_ap=idxB[:], num_idxs=gb * cb * P,
                        num_idxs_reg=gb * cb * P, elem_size=DOUT,
                        single_packet=False, queue_num=(q0 + 1) % 4)
                    nc.gpsimd.dma_gather(
                        out_ap=whdst[:, :, :], in_ap=whs_d[:, :],
                        idxs_ap=idxD[:], num_idxs=bc * P, num_idxs_reg=bc * P,
                        elem_size=DOUT, single_packet=False, queue_num=(q0 + 2) % 4)

                    s = spool.tile([P, bc, DOUT], F32, tag="s")
                    nc.vector.tensor_add(s[:], whsrc[:], whdst[:])
                    nc.scalar.activation(s[:], s[:], AF.Prelu, alpha=NSLOPE)
                    nc.vector.tensor_mul(s[:], s[:],
                                         a_rep[:].unsqueeze(1)
                                         .to_broadcast((P, bc, DOUT)))
                    e = spool.tile([P, bc], F32, tag="e")
                    nc.vector.tensor_reduce(e[:], s[:], axis=mybir.AxisListType.X,
                                            op=mybir.AluOpType.add)
                    ex = spool.tile([P, bc], F32, tag="ex")
                    nc.scalar.activation(ex[:], e[:], AF.Exp)

                    v = vpool.tile([P, bc, DOUT + 1], F16, tag="v")
                    nc.vector.tensor_mul(v[:, :, 0:DOUT], whsrc[:],
                                         ex[:].unsqueeze(2)
                                         .to_broadcast((P, bc, DOUT)))
                    nc.vector.tensor_copy(v[:, :, DOUT:DOUT + 1], ex[:].unsqueeze(2))

                    oh = opool.tile([P, bc, P], F16, tag="oh")
                    nc.vector.tensor_tensor(
                        oh[:],
                        dtl[:].unsqueeze(2).to_broadcast((P, bc, P)),
                        iota_f[:].unsqueeze(1).to_broadcast((P, bc, P)),
                        op=mybir.AluOpType.is_equal)

                    for tj in range(gb):
                        T = bi * gb + tj
                        ps = apool.tile([P, DOUT + 1], F32, tag="agg")
                        chunks = ([tj * ca + c for c in range(ca)]
                                  + [nA + tj * cb + c for c in range(cb)])
                        for k, c in enumerate(chunks):
                            nc.tensor.matmul(ps[:], lhsT=oh[:, c, :], rhs=v[:, c, :],
                                             start=(k == 0),
                                             stop=(k == len(chunks) - 1))
                        den = npool.tile([P, 1], F32, tag="den")
                        nc.vector.tensor_scalar_max(den[:], ps[:, DOUT:DOUT + 1], 1e-9)
                        rec = npool.tile([P, 1], F32, tag="rec")
                        nc.vector.reciprocal(rec[:], den[:])
                        ob = npool.tile([P, DOUT], F32, tag="ob")
                        nc.scalar.activation(ob[:], ps[:, 0:DOUT], AF.Sigmoid,
                                             scale=rec[:])
                        nc.sync.dma_start(out_d[T * P:(T + 1) * P, :], ob[:])

    nc.compile()
    return nc


_CACHE = {}


def kernel(x, W, b, a, edge_index):
    x = np.ascontiguousarray(np.asarray(x, dtype=np.float32))
    W = np.ascontiguousarray(np.asarray(W, dtype=np.float32))
    b = np.ascontiguousarray(np.asarray(b, dtype=np.float32))
    a = np.ascontiguousarray(np.asarray(a, dtype=np.float32))
    edge_index = np.asarray(edge_index)

    cfg, in_maps, meta = prepare(x, W, b, a, edge_index, gb_tiles=2)
    nc = _CACHE.get(cfg)
    if nc is None:
        nc = build(cfg)
        _CACHE[cfg] = nc

    from concourse.bass_utils import run_bass_kernel_spmd
    res = run_bass_kernel_spmd(nc, in_maps, core_ids=list(range(N_CORES)))
    parts = [res.results[c]["out"] for c in range(N_CORES)]
    return np.concatenate(parts, axis=0)[:meta["N"]].astype(np.float32)
